# revision 26
# baseline (speedup 1.0000x reference)
"""CoAttLayer Trainium2 kernel.

Data-parallel over batch: 64 batches -> 8 NeuronCores x 8 batches.
Per batch (T = N = 1024, d = 64, k = 128):
    L  = tanh(R @ Wl @ P^T)                      (T, N)
    Hp = tanh(Wp @ P^T + (Wr @ R^T) @ L)         (k, N)
    Hr = tanh(Wr @ R^T + (Wp @ P^T) @ L^T)       (k, T)
    Ap = softmax(whp @ Hp), Ar = softmax(whr @ Hr)
    out = [P^T @ Ap ; R^T @ Ar]                  (2d,)

Layout strategy: all d-contractions run on partitions 0-63 (R^T, P^T, A^T and
the transposed small weights all live there).  L is produced tile-wise in PSUM
(t on partitions), tanh'd by ScalarE straight into fp16 SBUF, and L^T is
produced by the DMA xbar transpose (fp16) so neither the PE nor the DVE pays
for the big transpose.  Big matmuls run in float32r (full PE rate at free-dim
512); the L-sized operands run in fp16.
"""

import numpy as np
from contextlib import ExitStack

B, T, N, D, K = 64, 1024, 1024, 64, 128
NCORES = 8
BL = B // NCORES  # batches per core

_CACHE = {}


def _build():
    import concourse.tile as tile
    from concourse import bacc, mybir
    from concourse.masks import make_identity

    f32 = mybir.dt.float32
    f32r = mybir.dt.float32r
    f16 = mybir.dt.float16
    Tanh = mybir.ActivationFunctionType.Tanh
    Exp = mybir.ActivationFunctionType.Exp

    nc = bacc.Bacc(trn_type="TRN2")

    rv = nc.dram_tensor("review_seq", (BL, T, D), f32r, kind="ExternalInput")
    po = nc.dram_tensor("post_seq", (BL, N, D), f32r, kind="ExternalInput")
    wl = nc.dram_tensor("Wl", (D, D), f32r, kind="ExternalInput")
    wr = nc.dram_tensor("Wr", (K, D), f32r, kind="ExternalInput")
    wp = nc.dram_tensor("Wp", (K, D), f32r, kind="ExternalInput")
    whr = nc.dram_tensor("whr", (1, K), f32, kind="ExternalInput")
    whp = nc.dram_tensor("whp", (1, K), f32, kind="ExternalInput")
    out = nc.dram_tensor("out", (BL, 2 * D), f32, kind="ExternalOutput")
    import os
    DBG = bool(int(os.environ.get("KBDBG", "0")))
    if DBG:
        dbg_lf = nc.dram_tensor("dbg_lf", (BL, 128, 8, 1024), f16, kind="ExternalOutput")
        dbg_lt = nc.dram_tensor("dbg_lt", (BL, 128, 8, 1024), f16, kind="ExternalOutput")
        dbg_hp = nc.dram_tensor("dbg_hp", (BL, 128, 1024), f16, kind="ExternalOutput")
        dbg_hr = nc.dram_tensor("dbg_hr", (BL, 128, 1024), f16, kind="ExternalOutput")
        dbg_ee = nc.dram_tensor("dbg_ee", (BL, 128, 16), f16, kind="ExternalOutput")

    NT = T // 128  # 8 t-tiles
    NN = N // 128  # 8 n-tiles

    with tile.TileContext(nc) as tc, ExitStack() as ctx:
        singles = ctx.enter_context(tc.tile_pool(name="singles", bufs=1))
        sb = ctx.enter_context(tc.tile_pool(name="sb", bufs=2))
        pa = ctx.enter_context(tc.tile_pool(name="pa", bufs=2, space="PSUM"))
        pb = ctx.enter_context(tc.tile_pool(name="pb", bufs=2, space="PSUM"))

        # ---- per-core constants -------------------------------------------
        ident32 = singles.tile([128, 128], f32)
        make_identity(nc, ident32)
        ident = singles.tile([128, 128], f32r)
        nc.vector.tensor_copy(ident, ident32)
        one11 = singles.tile([1, 1], f32)
        nc.vector.memset(one11, 1.0)
        ident16 = singles.tile([128, 128], f16)
        nc.vector.tensor_copy(ident16, ident32)

        wl_sb = singles.tile([64, 64], f32r)
        nc.sync.dma_start(out=wl_sb, in_=wl[:, :])
        wr_sb = singles.tile([128, 64], f32r)
        nc.sync.dma_start(out=wr_sb, in_=wr[:, :])
        wp_sb = singles.tile([128, 64], f32r)
        nc.sync.dma_start(out=wp_sb, in_=wp[:, :])
        whp_sb = singles.tile([1, 128], f32)
        nc.sync.dma_start(out=whp_sb, in_=whp[:, :])
        whr_sb = singles.tile([1, 128], f32)
        nc.sync.dma_start(out=whr_sb, in_=whr[:, :])

        # Wr^T, Wp^T on partitions 0-63; whp^T/whr^T as fp16 columns.
        ps_w = pa.tile([128, 1024], f32r, tag="pa")
        nc.tensor.transpose(ps_w[0:64, 0:128], wr_sb, ident)
        nc.tensor.transpose(ps_w[0:64, 128:256], wp_sb, ident)
        wrT = singles.tile([64, 128], f32r)
        nc.vector.tensor_copy(wrT, ps_w[0:64, 0:128])
        wpT = singles.tile([64, 128], f32r)
        nc.vector.tensor_copy(wpT, ps_w[0:64, 128:256])
        ps_wh = pa.tile([128, 2], f32, tag="pa")
        nc.tensor.transpose(ps_wh[0:128, 0:1], whp_sb, one11)
        nc.tensor.transpose(ps_wh[0:128, 1:2], whr_sb, one11)
        whT = singles.tile([128, 2], f16)
        nc.vector.tensor_copy(whT, ps_wh)

        # ---- per-batch pipeline -------------------------------------------
        for b in range(BL):
            # load R, P as 8 x (128, 64) t-tiles
            RP = sb.tile([128, NT, 64], f32r, tag="rp")
            PP = sb.tile([128, NN, 64], f32r, tag="pp")
            nc.sync.dma_start(out=RP, in_=rv[b, :, :].rearrange("(i p) d -> p i d", p=128))
            nc.sync.dma_start(out=PP, in_=po[b, :, :].rearrange("(i p) d -> p i d", p=128))

            # transpose inputs to d-on-partitions: Rt, Pt (64, 1024)
            ps_rt = pa.tile([128, 1024], f32r, tag="pa")
            for i in range(NT):
                nc.tensor.transpose(ps_rt[0:64, 128 * i:128 * (i + 1)], RP[:, i, :], ident)
            Rt = sb.tile([64, 1024], f32r, tag="rt")
            nc.vector.tensor_copy(Rt, ps_rt[0:64, :])

            ps_pt = pa.tile([128, 1024], f32r, tag="pa")
            for i in range(NN):
                nc.tensor.transpose(ps_pt[0:64, 128 * i:128 * (i + 1)], PP[:, i, :], ident)
            Pt = sb.tile([64, 1024], f32r, tag="pt")
            nc.vector.tensor_copy(Pt, ps_pt[0:64, :])

            # A^T = Wl^T @ R^T   (d', t) on partitions 0-63
            ps_at = pa.tile([128, 1024], f32, tag="pa")
            nc.tensor.matmul(ps_at[0:64, 0:512], wl_sb, Rt[:, 0:512], start=True, stop=True)
            nc.tensor.matmul(ps_at[0:64, 512:1024], wl_sb, Rt[:, 512:1024], start=True, stop=True)
            AT = sb.tile([64, 1024], f32r, tag="at")
            nc.vector.tensor_copy(AT, ps_at[0:64, :])

            # L tiles: L_i = tanh(A_i @ P^T) -> fp16 ; L^T via PE transposes
            Lf = sb.tile([128, NT, 1024], f16, tag="lf")
            LT = sb.tile([128, NN, 1024], f16, tag="lt")  # [p, j, t] = L[t, 128j+p]
            for i in range(NT):
                ps_l = pa.tile([128, 1024], f32, tag="pa")
                lhs = AT[:, 128 * i:128 * (i + 1)]
                nc.tensor.matmul(ps_l[:, 0:512], lhs, Pt[:, 0:512], start=True, stop=True)
                nc.tensor.matmul(ps_l[:, 512:1024], lhs, Pt[:, 512:1024], start=True, stop=True)
                nc.scalar.activation(Lf[:, i, :], ps_l, Tanh)
                ps_lt = pa.tile([128, NN, 128], f16, tag="pa")
                for j in range(NN):
                    nc.tensor.transpose(ps_lt[:, j, :], Lf[:, i, 128 * j:128 * (j + 1)], ident16)
                nc.vector.tensor_copy(LT[:, :, 128 * i:128 * (i + 1)], ps_lt)

            # G_r = Wr @ R^T (k, t), G_p = Wp @ P^T (k, n) -> fp16 + transposes
            ps_gr = pb.tile([128, 1024], f32, tag="pb")
            nc.tensor.matmul(ps_gr[:, 0:512], wrT, Rt[:, 0:512], start=True, stop=True)
            nc.tensor.matmul(ps_gr[:, 512:1024], wrT, Rt[:, 512:1024], start=True, stop=True)
            Gr16 = sb.tile([128, 1024], f16, tag="gr16")
            nc.vector.tensor_copy(Gr16, ps_gr)
            GrT = sb.tile([128, NT, 128], f16, tag="grt")
            ps_gtr = pb.tile([128, NT, 128], f16, tag="pb")
            for a in range(NT):
                nc.tensor.transpose(ps_gtr[:, a, :], Gr16[:, 128 * a:128 * (a + 1)], ident16)
            nc.vector.tensor_copy(GrT, ps_gtr)

            ps_gp = pb.tile([128, 1024], f32, tag="pb")
            nc.tensor.matmul(ps_gp[:, 0:512], wpT, Pt[:, 0:512], start=True, stop=True)
            nc.tensor.matmul(ps_gp[:, 512:1024], wpT, Pt[:, 512:1024], start=True, stop=True)
            Gp16 = sb.tile([128, 1024], f16, tag="gp16")
            nc.vector.tensor_copy(Gp16, ps_gp)
            GpT = sb.tile([128, NN, 128], f16, tag="gpt")
            ps_gtp = pb.tile([128, NN, 128], f16, tag="pb")
            for a in range(NN):
                nc.tensor.transpose(ps_gtp[:, a, :], Gp16[:, 128 * a:128 * (a + 1)], ident16)
            nc.vector.tensor_copy(GpT, ps_gtp)

            # Hp = tanh(G_p + sum_t G_r^T.T @ L)   (k, n)
            ps_hp = pb.tile([128, 1024], f32, tag="pb")
            nc.tensor.matmul(ps_hp[:, 0:512], wpT, Pt[:, 0:512], start=True, stop=False)
            nc.tensor.matmul(ps_hp[:, 512:1024], wpT, Pt[:, 512:1024], start=True, stop=False)
            for j in range(NT):
                nc.tensor.matmul(ps_hp[:, 0:512], GrT[:, j, :], Lf[:, j, 0:512],
                                 start=False, stop=(j == NT - 1))
                nc.tensor.matmul(ps_hp[:, 512:1024], GrT[:, j, :], Lf[:, j, 512:1024],
                                 start=False, stop=(j == NT - 1))
            Hp16 = sb.tile([128, 1024], f16, tag="hp16")
            nc.scalar.activation(Hp16, ps_hp, Tanh)

            # Hr = tanh(G_r + sum_n G_p^T.T @ L^T)   (k, t)
            ps_hr = pb.tile([128, 1024], f32, tag="pb")
            nc.tensor.matmul(ps_hr[:, 0:512], wrT, Rt[:, 0:512], start=True, stop=False)
            nc.tensor.matmul(ps_hr[:, 512:1024], wrT, Rt[:, 512:1024], start=True, stop=False)
            for j in range(NN):
                nc.tensor.matmul(ps_hr[:, 0:512], GpT[:, j, :], LT[:, j, 0:512],
                                 start=False, stop=(j == NN - 1))
                nc.tensor.matmul(ps_hr[:, 512:1024], GpT[:, j, :], LT[:, j, 512:1024],
                                 start=False, stop=(j == NN - 1))
            Hr16 = sb.tile([128, 1024], f16, tag="hr16")
            nc.scalar.activation(Hr16, ps_hr, Tanh)

            # logits^T: (n,1) and (t,1) per 128-chunk, then exp (no max-sub:
            # |logit| <= ||wh||_1 ~ 5, exp stays in fp16 range)
            ps_lg = pa.tile([128, 16], f32, tag="pa")
            for i in range(NN):
                nc.tensor.matmul(ps_lg[:, i:i + 1], Hp16[:, 128 * i:128 * (i + 1)],
                                 whT[:, 0:1], start=True, stop=True)
            for i in range(NT):
                nc.tensor.matmul(ps_lg[:, 8 + i:9 + i], Hr16[:, 128 * i:128 * (i + 1)],
                                 whT[:, 1:2], start=True, stop=True)
            ee = sb.tile([128, 16], f16, tag="ee")
            nc.scalar.activation(ee, ps_lg, Exp)

            # pooling rhs with an appended ones column -> unnormalized co + sum
            Pe = sb.tile([128, NN, 65], f16, tag="pe")
            nc.vector.tensor_copy(Pe[:, :, 0:64], PP)
            nc.vector.memset(Pe[:, :, 64:65], 1.0)
            Re = sb.tile([128, NT, 65], f16, tag="re")
            nc.vector.tensor_copy(Re[:, :, 0:64], RP)
            nc.vector.memset(Re[:, :, 64:65], 1.0)

            ps_co = pa.tile([128, 1024], f32, tag="pa")
            for j in range(NN):
                nc.tensor.matmul(ps_co[0:1, 0:65], ee[:, j:j + 1], Pe[:, j, :],
                                 start=(j == 0), stop=(j == NN - 1))
            for j in range(NT):
                nc.tensor.matmul(ps_co[0:1, 512:577], ee[:, 8 + j:9 + j], Re[:, j, :],
                                 start=(j == 0), stop=(j == NT - 1))

            if DBG:
                nc.sync.dma_start(out=dbg_lf[b], in_=Lf)
                nc.sync.dma_start(out=dbg_lt[b], in_=LT)
                nc.sync.dma_start(out=dbg_hp[b], in_=Hp16)
                nc.sync.dma_start(out=dbg_hr[b], in_=Hr16)
                nc.sync.dma_start(out=dbg_ee[b], in_=ee)
            rinv = sb.tile([1, 2], f32, tag="rinv")
            nc.vector.reciprocal(rinv[0:1, 0:1], ps_co[0:1, 64:65])
            nc.vector.reciprocal(rinv[0:1, 1:2], ps_co[0:1, 576:577])
            ob = sb.tile([1, 128], f32, tag="ob")
            nc.vector.tensor_scalar_mul(ob[0:1, 0:64], ps_co[0:1, 0:64], rinv[0:1, 0:1])
            nc.vector.tensor_scalar_mul(ob[0:1, 64:128], ps_co[0:1, 512:576], rinv[0:1, 1:2])
            nc.sync.dma_start(out=out[b:b + 1, :], in_=ob)

    nc.compile()
    return nc


def get_nc():
    if "nc" not in _CACHE:
        _CACHE["nc"] = _build()
    return _CACHE["nc"]


def make_in_maps(inputs):
    R = np.ascontiguousarray(inputs["review_seq"], dtype=np.float32)
    P = np.ascontiguousarray(inputs["post_seq"], dtype=np.float32)
    w = {
        "Wl": np.ascontiguousarray(inputs["Wl"], dtype=np.float32),
        "Wr": np.ascontiguousarray(inputs["Wr"], dtype=np.float32),
        "Wp": np.ascontiguousarray(inputs["Wp"], dtype=np.float32),
        "whr": np.ascontiguousarray(inputs["whr"], dtype=np.float32),
        "whp": np.ascontiguousarray(inputs["whp"], dtype=np.float32),
    }
    in_maps = []
    for c in range(NCORES):
        m = {
            "review_seq": np.ascontiguousarray(R[c * BL:(c + 1) * BL]),
            "post_seq": np.ascontiguousarray(P[c * BL:(c + 1) * BL]),
        }
        m.update(w)
        in_maps.append(m)
    return in_maps


def run(inputs, trace=False):
    from concourse.bass_utils import run_bass_kernel_spmd

    nc = get_nc()
    res = run_bass_kernel_spmd(nc, make_in_maps(inputs),
                               core_ids=list(range(NCORES)), trace=trace)
    outp = np.concatenate([r["out"] for r in res.results], axis=0)
    return outp.astype(np.float32), res


def kernel(**inputs) -> np.ndarray:
    outp, _ = run(inputs, trace=False)
    return outp
